# revision 45
# baseline (speedup 1.0000x reference)
"""Multi-head attention (B=4, N=2048, C=768, H=12) on 8 Trainium2 NeuronCores.

Sharding: core c = (batch b = c//2, head-group g = c%2 of 6 heads).
Each core: qkv projection for its (b, g), attention for 6 heads, partial
output projection against w_proj[:, g-cols]. Host sums the two partial
projections per batch, adds bias, transposes. No collectives.

All SBUF tensors bf16 (psum f32). The ACT engine's exp stream (192 tiles
of [128,1024], ~199us) is the critical resource; everything else is
scheduled around keeping it saturated:

  scores sT[k,q] per head: lhsT = k chunk [64,128], rhs = q [64,512]x2
    -> psum [128,1024] (2 banks, "sc" pool bufs=2), exp on ACT -> eT bf16.
  PV is FLIPPED to use all 128 output partitions: out[q,d] with
    lhsT = eT q-block [128,128] (stationary), rhs = v chunk [128,64]
    -> psum acc [128, 8*64] = exactly 1 bank/head ("pva" bufs=2); the
    softmax denominator accumulates via a ones-column rhs into a shared
    "den" bank [128,2,8]. PSUM groups are BANK-granular (start zeroes the
    whole 2KB zero-region, first touch overwrites): one start/stop pair
    per bank per segment, any emission order in between.
  normalize on DVE (per-partition reciprocal broadcast), then the
    [q,d]->[d,n] transpose rides the idle DMA xbar (dma_start_transpose,
    8 tiles x 14ns per [128,128] block) straight into out_h [f,n].
  qkv/vT/proj matmuls are deferred into per-segment thunk "windows" that
    trickle into the PE stream (~<=850ns/iteration) so the in-order PE
    never delays a score matmul; deferred PV chunks (any-order psum
    accumulation) absorb the remaining slack.

PSUM banks: sc 2x2 + pva 2x1 + den 1 + win 1 = 8.
"""

import sys

for _p in ("/opt/trn_rl_repo", "/root/.axon_site/_ro/trn_rl_repo"):
    if _p not in sys.path:
        sys.path.insert(0, _p)

import numpy as np
import ml_dtypes

import concourse.bass as bass
import concourse.bacc as bacc
import concourse.mybir as mybir
import concourse.tile as tile
from concourse.bass_utils import run_bass_kernel_spmd

B, N, C = 4, 2048, 768
H, D = 12, 64
HG = 6          # heads per core
P = 128
NCORES = 8
CK = C // P     # 6 contraction chunks for qkv
NT = N // P     # 16 token chunks
QG = 2          # q-groups of 1024
QW = N // QG    # 1024
NQB = QW // P   # 8 q-blocks per q-group
SCALE = D ** -0.5
KB = HG * D // P  # 3: first k block index offset in qk_sb

B_DT = mybir.dt.bfloat16
NP_BF = ml_dtypes.bfloat16

PRE = 8         # score-chunk lookahead of the exp stream
LAG = 3         # PV chunks trail the pump by this many iterations
ET_BUFS = 24

_CACHED_NC = None


def build_nc():
    nc = bacc.Bacc("TRN2", target_bir_lowering=False, debug=False, num_devices=NCORES)
    f32 = mybir.dt.float32

    xT = nc.declare_dram_parameter("xT", [P, CK, N], B_DT, isOutput=False)
    wqk = nc.declare_dram_parameter("wqk", [P, CK, 2 * HG * D], B_DT, isOutput=False)
    wv = nc.declare_dram_parameter("wv", [P, CK, HG * D], B_DT, isOutput=False)
    wp = nc.declare_dram_parameter("wp", [P, KB, C], B_DT, isOutput=False)
    out = nc.declare_dram_parameter("out", [C, N], B_DT, isOutput=True)
    # fc0+fc1 partials for the ACT-finished tail blocks; host adds them
    pout = nc.declare_dram_parameter("pout", [P, 3, QW], B_DT, isOutput=True)

    with tile.TileContext(nc) as tc:
        with (
            tc.tile_pool(name="big", bufs=1) as big,
            tc.tile_pool(name="et", bufs=ET_BUFS) as etp,
            tc.tile_pool(name="asb", bufs=2) as asbp,
            tc.tile_pool(name="rcp", bufs=2) as recp,
            tc.tile_pool(name="stg", bufs=6) as stg,
            tc.tile_pool(name="psc", bufs=2, space="PSUM") as psc,
            tc.tile_pool(name="pva", bufs=2, space="PSUM") as pva,
            tc.tile_pool(name="pdn", bufs=1, space="PSUM") as pdn,
            tc.tile_pool(name="pwn", bufs=1, space="PSUM") as pwn,
        ):
            # ---------------- loads ----------------
            # critical-path first: the upfront q0/k0 chains (and so the first
            # exp) only need wqk blocks 0 and 3 and xT columns 0:1024
            xT_sb = big.tile([P, CK, N], B_DT)
            wqk_sb = big.tile([P, CK, 2 * HG * D], B_DT)
            wv_sb = big.tile([P, CK, HG * D], B_DT)
            # upfront only needs wqk blocks 0 (q0) and 3 (k0): one strided DMA
            # per chunk; xT halves issue from the Pool SWDGE path so the two
            # descriptor-generation chains run in parallel
            wqk_v = wqk_sb.rearrange("p k (b c) -> p k b c", c=P)
            wqk_d = wqk.rearrange("p k (b c) -> p k b c", c=P)
            for kc in range(CK):
                nc.sync.dma_start(wqk_v[:, kc, 0 : KB + 1 : KB, :],
                                  wqk_d[:, kc, 0 : KB + 1 : KB, :])
                nc.gpsimd.dma_start(xT_sb[:, kc, 0:QW], xT[:, kc, 0:QW])
            for kc in range(CK):
                nc.sync.dma_start(wqk_v[:, kc, 1:KB, :], wqk_d[:, kc, 1:KB, :])
                nc.sync.dma_start(wqk_v[:, kc, KB + 1 :, :], wqk_d[:, kc, KB + 1 :, :])
            nc.sync.dma_start(wv_sb, wv[:, :, :])
            for kc in range(CK):
                nc.sync.dma_start(xT_sb[:, kc, QW:N], xT[:, kc, QW:N])
            wp_sb = big.tile([P, KB, C], B_DT)
            nc.sync.dma_start(wp_sb, wp[:, :, :])

            # warm the ACT exp table during the load phase
            warm = recp.tile([1, 32], f32, tag="warm")
            nc.vector.memset(warm, 0.0)
            nc.scalar.activation(warm, warm, mybir.ActivationFunctionType.Exp,
                                 bias=0.0, scale=1.0)
            # keep the PE busy through the load phase so the p-state ramp hits
            # full clock before the first real matmuls (results are unused)
            pe_warm = pwn.tile([P, 512], f32, tag="win", name="pe_warm")
            for _ in range(48):
                nc.tensor.matmul(
                    pe_warm[:, 0:64],
                    lhsT=wqk_sb[:, 0, 0:P],
                    rhs=xT_sb[:, 0, 0:64],
                    start=True,
                    stop=True,
                )


            # qk[o, n]: blocks 0-2 = q heads, 3-5 = k heads (head h at
            # partitions (h%2)*64 of block h//2)
            qk_sb = big.tile([P, 2 * KB, N], B_DT)
            # vT[n, f] with per-head ones column
            vT_sb = big.tile([P, NT, HG * (D + 1)], B_DT)
            ones_view = vT_sb.rearrange("p n (h s) -> p n h s", s=D + 1)[:, :, :, D : D + 1]
            nc.vector.memset(ones_view, 1.0)
            # attention outputs [f, n]: pair p on block p, (qg, qb, 128)
            out_h = big.tile([P, KB, QG, NQB, P], B_DT)
            # fc0+fc1 partial projections; the finish step is fc2 + add
            partial_nh0 = big.tile([P, C // P, QW], B_DT)
            partial_nh1 = big.tile([P, C // P, QW], B_DT)

            # ---------------- helpers ----------------
            def emit_sc_block(ps, ot, nh, kc):
                for i in range(2):
                    nc.tensor.matmul(
                        ps[:, i * 512 : (i + 1) * 512],
                        lhsT=wqk_sb[:, kc, ot * P : (ot + 1) * P],
                        rhs=xT_sb[:, kc, nh * QW + i * 512 : nh * QW + (i + 1) * 512],
                        start=(kc == 0),
                        stop=(kc == CK - 1),
                    )

            def win_chain_thunks(ot, nh, half):
                """qk half-block [P,512] on the serial win bank: 3 thunks."""
                state = {}

                def start():
                    state["w"] = pwn.tile([P, 512], f32, tag="win",
                                          name=f"w{ot}_{nh}_{half}")
                    for kc in range(3):
                        nc.tensor.matmul(
                            state["w"],
                            lhsT=wqk_sb[:, kc, ot * P : (ot + 1) * P],
                            rhs=xT_sb[:, kc, nh * QW + half * 512 : nh * QW + (half + 1) * 512],
                            start=(kc == 0),
                            stop=False,
                        )

                def mid():
                    for kc in range(3, CK):
                        nc.tensor.matmul(
                            state["w"],
                            lhsT=wqk_sb[:, kc, ot * P : (ot + 1) * P],
                            rhs=xT_sb[:, kc, nh * QW + half * 512 : nh * QW + (half + 1) * 512],
                            start=False,
                            stop=(kc == CK - 1),
                        )

                def fin():
                    nc.vector.tensor_copy(
                        qk_sb[:, ot, nh * QW + half * 512 : nh * QW + (half + 1) * 512],
                        state["w"],
                    )

                return [start, mid, fin]

            def vt_thunks(nt):
                """vT group for token chunk nt on the win bank: 2 thunks."""
                state = {}

                def a():
                    state["w"] = pwn.tile([P, 512], f32, tag="win", name=f"vt{nt}")
                    for kc in range(3):
                        nc.tensor.matmul(
                            state["w"][:, 0 : HG * D],
                            lhsT=xT_sb[:, kc, nt * P : (nt + 1) * P],
                            rhs=wv_sb[:, kc, :],
                            start=(kc == 0),
                            stop=False,
                        )

                def b():
                    for kc in range(3, CK):
                        nc.tensor.matmul(
                            state["w"][:, 0 : HG * D],
                            lhsT=xT_sb[:, kc, nt * P : (nt + 1) * P],
                            rhs=wv_sb[:, kc, :],
                            start=False,
                            stop=(kc == CK - 1),
                        )
                    nc.vector.tensor_copy(
                        vT_sb.rearrange("p n (h s) -> p n h s", s=D + 1)[:, nt, :, 0:D],
                        state["w"][:, 0 : HG * D].rearrange("p (h s) -> p h s", s=D),
                    )

                return [a, b]

            def proj_partial_thunks(ot, half):
                """fc0+fc1 of proj(ot, nh=0) on the win bank -> bf16 partial."""
                state = {}

                def a():
                    state["w"] = pwn.tile([P, 512], f32, tag="win",
                                          name=f"pp{ot}_{half}")
                    for fc in range(2):
                        nc.tensor.matmul(
                            state["w"],
                            lhsT=wp_sb[:, fc, ot * P : (ot + 1) * P],
                            rhs=out_h[:, fc, 0, half * 4 : (half + 1) * 4, :],
                            start=(fc == 0),
                            stop=(fc == 1),
                        )

                def b():
                    nc.vector.tensor_copy(
                        partial_nh0[:, ot, half * 512 : (half + 1) * 512], state["w"]
                    )

                return [a, b]

            def partial_nh1_thunks(ot, half):
                """fc0+fc1 of proj(ot, nh=1) on the win bank -> bf16 partial."""
                state = {}

                def a():
                    state["w"] = pwn.tile([P, 512], f32, tag="win",
                                          name=f"pq{ot}_{half}")
                    for fc in range(2):
                        nc.tensor.matmul(
                            state["w"],
                            lhsT=wp_sb[:, fc, ot * P : (ot + 1) * P],
                            rhs=out_h[:, fc, 1, half * 4 : (half + 1) * 4, :],
                            start=(fc == 0),
                            stop=(fc == 1),
                        )

                def b():
                    nc.vector.tensor_copy(
                        partial_nh1[:, ot, half * 512 : (half + 1) * 512], state["w"]
                    )

                return [a, b]

            def fin_nh0_thunks(ot):
                """fc2 + fused partial add for nh=0, fully on the win bank so
                it overlaps the final pump (the sc banks stay with the exps).
                Two [P,512] halves sharing one staging tile."""
                state = {}

                def mk(half):
                    def a():
                        state[half] = pwn.tile([P, 512], f32, tag="win",
                                               name=f"fh{ot}_{half}")
                        nc.tensor.matmul(
                            state[half],
                            lhsT=wp_sb[:, KB - 1, ot * P : (ot + 1) * P],
                            rhs=out_h[:, KB - 1, 0, half * 4 : (half + 1) * 4, :],
                            start=True,
                            stop=True,
                        )

                    def b():
                        if "so" not in state:
                            state["so"] = stg.tile([P, QW], B_DT, tag="so",
                                                   name=f"fso{ot}_0")
                        so = state["so"]
                        nc.vector.tensor_tensor(
                            so[:, half * 512 : (half + 1) * 512],
                            state[half],
                            partial_nh0[:, ot, half * 512 : (half + 1) * 512],
                            mybir.AluOpType.add,
                        )
                        nc.sync.dma_start(
                            out[ot * P : (ot + 1) * P,
                                half * 512 : (half + 1) * 512],
                            so[:, half * 512 : (half + 1) * 512],
                        )

                    return [a, b]

                return mk(0) + mk(1)

            def fin_nh1_tail(ot):
                """tail: fc2 for nh=1. Blocks 0-2 finish on ACT (copy; the
                host adds the DMA'd partial), blocks 3-5 on DVE (fused add);
                the two chains run in parallel across 5 free PSUM slots."""
                ps = psc.tile([P, QW], f32, tag="sc", name=f"pf{ot}_1")
                for i in range(2):
                    nc.tensor.matmul(
                        ps[:, i * 512 : (i + 1) * 512],
                        lhsT=wp_sb[:, KB - 1, ot * P : (ot + 1) * P],
                        rhs=out_h[:, KB - 1, 1, i * 4 : (i + 1) * 4, :],
                        start=True,
                        stop=True,
                    )
                so = stg.tile([P, QW], B_DT, tag="so", name=f"fso{ot}_1")
                if ot < 3:
                    nc.scalar.copy(so, ps)
                else:
                    nc.vector.tensor_tensor(so, ps, partial_nh1[:, ot],
                                            mybir.AluOpType.add)
                nc.sync.dma_start(out[ot * P : (ot + 1) * P, QW:N], so)

            def fin_nh1_halves(ot, pool, tag):
                """same, but as two [P,512] groups on a borrowed bank pool."""
                so = stg.tile([P, QW], B_DT, tag="so", name=f"fso{ot}_1")
                for i in range(2):
                    w = pool.tile([P, 512], f32, tag=tag, name=f"pf{ot}_1_{i}")
                    nc.tensor.matmul(
                        w,
                        lhsT=wp_sb[:, KB - 1, ot * P : (ot + 1) * P],
                        rhs=out_h[:, KB - 1, 1, i * 4 : (i + 1) * 4, :],
                        start=True,
                        stop=True,
                    )
                    sl = slice(i * 512, (i + 1) * 512)
                    if ot < 3:
                        nc.scalar.copy(so[:, sl], w)
                    else:
                        nc.vector.tensor_tensor(so[:, sl], w,
                                                partial_nh1[:, ot, sl],
                                                mybir.AluOpType.add)
                nc.sync.dma_start(out[ot * P : (ot + 1) * P, QW:N], so)

            # ---------------- score pump ----------------
            segs = [(p_, qg) for p_ in range(HG // 2) for qg in range(QG)]
            score_queue = [(p_, qg, ch) for (p_, qg) in segs for ch in range(NT)]
            et_tiles = {}
            qpos = [0]

            def emit_scores(sp, sqg, ch):
                for e in range(2):
                    base = e * D
                    ps = psc.tile([P, QW], f32, tag="sc", name=f"s{sp}_{sqg}_{ch}_{e}")
                    for i in range(2):
                        nc.tensor.matmul(
                            ps[:, i * 512 : (i + 1) * 512],
                            lhsT=qk_sb[base : base + D, KB + sp, ch * P : (ch + 1) * P],
                            rhs=qk_sb[base : base + D, sp, sqg * QW + i * 512 : sqg * QW + (i + 1) * 512],
                            start=True,
                            stop=True,
                        )
                    eT = etp.tile([P, QW], B_DT, tag="et", name=f"e{sp}_{sqg}_{ch}_{e}")
                    nc.scalar.activation(
                        eT, ps, mybir.ActivationFunctionType.Exp,
                        bias=0.0, scale=float(SCALE),
                    )
                    et_tiles[(sp, sqg, ch, e)] = eT

            def pump_scores(n):
                for _ in range(n):
                    if qpos[0] < len(score_queue):
                        emit_scores(*score_queue[qpos[0]])
                        qpos[0] += 1

            # ---------------- upfront qkv ----------------
            # wave 1 on the sc banks, kc-outer so matmuls chase the DMA
            sc_q = psc.tile([P, QW], f32, tag="sc", name="up_q")
            sc_k = psc.tile([P, QW], f32, tag="sc", name="up_k")
            for kc in range(CK):
                emit_sc_block(sc_q, 0, 0, kc)
                emit_sc_block(sc_k, KB, 0, kc)
            # the first scores gate on these: one copy on the idle ACT engine
            nc.scalar.copy(qk_sb[:, 0, 0:QW], sc_q)
            nc.vector.tensor_copy(qk_sb[:, KB, 0:QW], sc_k)

            # wave 2 on the serial win bank, interleaved with the first pumps
            pump_scores(2)
            up_chains = [
                win_chain_thunks(KB, 1, 0),   # k0 nh1 lo  (needed (0,0) it 0)
                win_chain_thunks(KB, 1, 1),   # k0 nh1 hi
                win_chain_thunks(0, 1, 0),    # q0 nh1 lo  (needed (0,0) it 8)
                win_chain_thunks(0, 1, 1),    # q0 nh1 hi
            ]
            for chain in up_chains:
                for th in chain:
                    th()
                pump_scores(1)
            # PRE consumed: 2 + 4 = 6 pumps so far
            pump_scores(PRE - 6)

            # ---------------- windows ----------------
            windows = {
                (0, 0): [t for nt in range(NT) for t in vt_thunks(nt)],
                (0, 1): [t for args in ((1, 0, 0), (1, 0, 1), (KB + 1, 0, 0),
                                        (KB + 1, 0, 1), (KB + 1, 1, 0), (KB + 1, 1, 1))
                         for t in win_chain_thunks(*args)],
                (1, 0): [t for args in ((1, 1, 0), (1, 1, 1))
                         for t in win_chain_thunks(*args)],
                (1, 1): [t for args in ((2, 0, 0), (2, 0, 1), (KB + 2, 0, 0),
                                        (KB + 2, 0, 1), (KB + 2, 1, 0), (KB + 2, 1, 1))
                         for t in win_chain_thunks(*args)]
                        + [t for ot in range(4) for half in range(2)
                           for t in proj_partial_thunks(ot, half)],
                (2, 0): [t for args in ((2, 1, 0), (2, 1, 1))
                         for t in win_chain_thunks(*args)]
                        + [t for ot in range(4, C // P) for half in range(2)
                           for t in proj_partial_thunks(ot, half)]
                        + [t for ot in range(C // P) for half in range(2)
                           for t in partial_nh1_thunks(ot, half)],
                (2, 1): [t for ot in range(C // P) for t in fin_nh0_thunks(ot)],
            }
            # thunks per iteration: (gate_iter, count)
            win_rate = {(0, 0): (0, 2), (0, 1): (2, 2), (1, 0): (2, 2),
                        (1, 1): (2, 3), (2, 0): (2, 3), (2, 1): (0, 2)}

            # ---------------- attention ----------------
            last_eT = [None]

            def emit_pv(sp, sqg, ch, acc2, den, cnt):
                for e in range(2):
                    h = 2 * sp + e
                    eT = et_tiles.pop((sp, sqg, ch, e))
                    last_eT[0] = eT
                    for qb in range(NQB):
                        lhs = eT[:, qb * P : (qb + 1) * P]
                        nc.tensor.matmul(
                            acc2[e][:, qb * D : (qb + 1) * D],
                            lhsT=lhs,
                            rhs=vT_sb[:, ch, h * (D + 1) : h * (D + 1) + D],
                            start=(cnt["a%d" % e] == 0),
                            stop=(cnt["a%d" % e] == NT * NQB - 1),
                        )
                        cnt["a%d" % e] += 1
                        nc.tensor.matmul(
                            den[:, e, qb : qb + 1],
                            lhsT=lhs,
                            rhs=vT_sb[:, ch, h * (D + 1) + D : (h + 1) * (D + 1)],
                            start=(cnt["d"] == 0),
                            stop=(cnt["d"] == 2 * NT * NQB - 1),
                        )
                        cnt["d"] += 1

            for sp, sqg in segs:
                last_seg = (sp, sqg) == segs[-1]
                if last_seg:
                    # partial blocks 0-2 finished in the previous segment's
                    # window; ship them to the host now, off the tail's queue
                    nc.sync.dma_start(pout[:, :, :], partial_nh1[:, 0:3, :])
                thunks = list(windows.get((sp, sqg), []))
                gate, rate = win_rate[(sp, sqg)]
                lag = 0 if last_seg else LAG
                acc2 = [
                    pva.tile([P, NQB * D], f32, tag="pva", name=f"a{sp}_{sqg}_{e}")
                    for e in range(2)
                ]
                den = pdn.tile([P, 2, NQB], f32, tag="den", name=f"dn{sp}_{sqg}")
                cnt = {"a0": 0, "a1": 0, "d": 0}
                backlog = []
                for ch in range(NT):
                    pump_scores(1)
                    backlog.append(ch)
                    if last_seg and ch < gate:
                        # PV immediately while the pump still runs; from the
                        # gate on, all thunks flush first (they are not
                        # exp-gated) so the exp-gated PVs don't serialize the
                        # in-order PE queue at ACT cadence
                        while backlog and backlog[0] <= ch - lag:
                            emit_pv(sp, sqg, backlog.pop(0), acc2, den, cnt)
                    if ch >= gate:
                        for _ in range(rate):
                            if thunks:
                                thunks.pop(0)()
                    while backlog and backlog[0] <= ch - lag:
                        emit_pv(sp, sqg, backlog.pop(0), acc2, den, cnt)
                while backlog:
                    emit_pv(sp, sqg, backlog.pop(0), acc2, den, cnt)
                while thunks:
                    thunks.pop(0)()

                # normalize then transpose via the DMA xbar (both off the PE);
                # the final segment's norm splits across DVE + GpSimd to
                # shorten the tail's serial chain
                rec = recp.tile([P, 2, NQB], f32, tag="rec", name=f"r{sp}_{sqg}")
                nc.vector.reciprocal(rec, den)
                a_sb = asbp.tile([P, NQB, P], B_DT, tag="asb", name=f"ab{sp}_{sqg}")
                # the last segment norms/transposes in 4-qb halves so the
                # tail's first fc2 matmuls start one transpose earlier
                halves = ((0, NQB // 2), (NQB // 2, NQB)) if last_seg else ((0, NQB),)
                for q0, q1 in halves:
                    for e in range(2):
                        nc.vector.tensor_tensor(
                            a_sb.rearrange("p q (e d) -> p q e d", e=2)[:, q0:q1, e, :],
                            acc2[e].rearrange("p (q d) -> p q d", d=D)[:, q0:q1],
                            rec[:, e, q0:q1].rearrange("p (q o) -> p q o", o=1).broadcast_to([P, q1 - q0, D]),
                            mybir.AluOpType.mult,
                        )
                    nc.sync.dma_start_transpose(
                        out_h[:, sp, sqg, q0:q1, :],
                        a_sb[:, q0:q1, :].rearrange("p a b -> p (a b)"),
                    )

            # ---------------- proj tail (nh=1): fc2 + fused partial add ----
            # keep the PE p-state warm through the norm/transpose bubble:
            # these read the final eT so they cannot be scheduled early
            pe_warm2 = pwn.tile([P, 512], f32, tag="win", name="pe_warm2")
            for _ in range(10):
                nc.tensor.matmul(
                    pe_warm2,
                    lhsT=last_eT[0][:, 0:P],
                    rhs=last_eT[0][:, 0:512],
                    start=True,
                    stop=True,
                )
            # interleave ACT-finished (0-2) and DVE-finished (3-5) blocks so
            # both finish chains start immediately, spread over 5 psum slots
            fin_nh1_tail(0)
            fin_nh1_tail(3)
            fin_nh1_halves(1, pva, "pva")
            fin_nh1_halves(4, pwn, "win")
            fin_nh1_tail(2)
            fin_nh1_tail(5)
    nc.compile()
    return nc


def _get_nc():
    global _CACHED_NC
    if _CACHED_NC is None:
        _CACHED_NC = build_nc()
    return _CACHED_NC


def shard_inputs(x, w_qkv, w_proj):
    """Build per-core input maps from full inputs (all bf16, pre-tiled)."""
    in_maps = []
    for c in range(NCORES):
        b, g = divmod(c, 2)
        r = slice(HG * D * g, HG * D * (g + 1))

        def ptile(m):
            return np.ascontiguousarray(
                m.reshape(m.shape[0] // P, P, m.shape[1]).transpose(1, 0, 2)
            ).astype(NP_BF)

        xT_ = ptile(x[b].T)
        wq = w_qkv[r]
        wk = w_qkv[C + HG * D * g : C + HG * D * (g + 1)]
        wv_ = w_qkv[2 * C + HG * D * g : 2 * C + HG * D * (g + 1)]
        wqk_ = ptile(np.concatenate([wq, wk], axis=0).T)
        wvT = ptile(wv_.T)
        wpT = ptile(w_proj[:, r].T)
        in_maps.append({"xT": xT_, "wqk": wqk_, "wv": wvT, "wp": wpT})
    return in_maps


def run(x, w_qkv, w_proj, b_proj, trace=False):
    nc = _get_nc()
    in_maps = shard_inputs(x, w_qkv, w_proj)
    try:
        res = run_bass_kernel_spmd(nc, in_maps, list(range(NCORES)), trace=trace)
    except Exception:
        res = run_bass_kernel_spmd(nc, in_maps, list(range(NCORES)), trace=trace)
    y = np.empty((B, N, C), np.float32)
    for b in range(B):
        part = np.zeros((C, N), np.float32)
        for g in range(2):
            r = res.results[2 * b + g]
            o = r["out"].astype(np.float32)
            po = r["pout"].astype(np.float32)  # [P, 3, QW]
            for ot in range(3):
                o[ot * P : (ot + 1) * P, QW:N] += po[:, ot, :]
            part += o
        y[b] = part.T + b_proj.astype(np.float32)
    return y, res


def kernel(x, w_qkv, w_proj, b_proj):
    x = np.asarray(x, dtype=np.float32)
    w_qkv = np.asarray(w_qkv, dtype=np.float32)
    w_proj = np.asarray(w_proj, dtype=np.float32)
    b_proj = np.asarray(b_proj, dtype=np.float32)
    y, _ = run(x, w_qkv, w_proj, b_proj, trace=False)
    return y
